# revision 9
# baseline (speedup 1.0000x reference)
"""Trainium2 Bass kernel for nn_EncoDecLSTM (B=256, T=512, F=64, U=128).

Strategy:
  - Data-parallel over batch: 8 cores x 32 batch elements each.
  - Feature-major activations [U=128 partitions, batch] everywhere; no
    transposes anywhere in the recurrence.
  - Encoder input projection + biases folded into PE PSUM accumulation
    (ones-row augmented x, mask-matmul for decoder bias) so the serial
    critical path per step is: 4 h-matmuls -> sigmoid ACT (all gates) ->
    3 fused DVE ops -> sigmoid ACT -> 1 fused DVE op.
  - tanh computed via tanh(x) = 2*sigmoid(2x) - 1 with the *2 baked into
    weights; hidden state stored as h~ = h/2 with the *2 compensation baked
    into every consumer weight matrix (enc_rk, dec_k+dec_rk, w1).
  - Decoder feeds its own output, and out == dh always, so dec_k + dec_rk
    collapse into one weight matrix.
  - Dense head (relu(seq@w1+b1)@w2+b2) runs on-chip after the decoder.
"""

import numpy as np

B, T, F, U = 256, 512, 64, 128
NCORES = 8
BL = B // NCORES           # 32 batch per core
ZCH = 4                    # z PSUM chunk (timesteps per PSUM bank)

_CACHE = {}


def _build_program(T_=T, dbg=False, ncores=NCORES):
    import concourse.bacc as bacc
    import concourse.tile as tile
    from concourse import mybir

    dt = mybir.dt.float32
    Sig = mybir.ActivationFunctionType.Sigmoid
    Relu = mybir.ActivationFunctionType.Relu
    sub = mybir.AluOpType.subtract
    mul = mybir.AluOpType.mult
    add = mybir.AluOpType.add

    XCH = min(16, T_)      # x DMA chunk (timesteps)

    nc = bacc.Bacc("TRN2", target_bir_lowering=False, debug=False,
                   num_devices=ncores)

    x_d = nc.dram_tensor("x", [F + 1, T_, BL], dt, kind="ExternalInput").ap()
    wx_d = nc.dram_tensor("wx", [4, F + 1, U], dt, kind="ExternalInput").ap()
    whe_d = nc.dram_tensor("whe", [U, 4 * U], dt, kind="ExternalInput").ap()
    whd_d = nc.dram_tensor("whd", [U, 4 * U], dt, kind="ExternalInput").ap()
    bdec_d = nc.dram_tensor("bdec", [4, U], dt, kind="ExternalInput").ap()
    maskc_d = nc.dram_tensor("maskc", [4, ZCH * 4 * BL], dt,
                             kind="ExternalInput").ap()
    w1_d = nc.dram_tensor("w1", [U, U], dt, kind="ExternalInput").ap()
    b1_d = nc.dram_tensor("b1", [U, 1], dt, kind="ExternalInput").ap()
    w2_d = nc.dram_tensor("w2", [U, F], dt, kind="ExternalInput").ap()
    b2t_d = nc.dram_tensor("b2t", [1, 8 * F], dt, kind="ExternalInput").ap()
    ones_d = nc.dram_tensor("ones", [1, BL], dt, kind="ExternalInput").ap()
    y_d = nc.dram_tensor("y", [BL, T_ * F], dt, kind="ExternalOutput").ap()
    if dbg:
        seqdbg_d = nc.dram_tensor("seqdbg", [U, T_ * BL], dt,
                                  kind="ExternalOutput").ap()
        henc_d = nc.dram_tensor("henc", [U, BL], dt,
                                kind="ExternalOutput").ap()
        cenc_d = nc.dram_tensor("cenc", [U, BL], dt,
                                kind="ExternalOutput").ap()

    NZ = T_ // ZCH         # z-chunks per phase
    NXC = T_ // XCH        # x DMA chunks

    with tile.TileContext(nc) as tc, \
         tc.tile_pool(name="consts", bufs=1) as consts, \
         tc.tile_pool(name="xpool", bufs=1) as xpool, \
         tc.tile_pool(name="seqp", bufs=1) as seqp, \
         tc.tile_pool(name="zp", bufs=3, space="PSUM") as zp, \
         tc.tile_pool(name="gp", bufs=3) as gp, \
         tc.tile_pool(name="cp", bufs=3) as cp, \
         tc.tile_pool(name="scp", bufs=3) as scp, \
         tc.tile_pool(name="hp", bufs=3) as hp, \
         tc.tile_pool(name="tmp", bufs=3) as tmp, \
         tc.tile_pool(name="dps", bufs=2, space="PSUM") as dps, \
         tc.tile_pool(name="ops", bufs=2, space="PSUM") as ops, \
         tc.tile_pool(name="dsb", bufs=2) as dsb:

        # ---- constants into SBUF ----
        wx_sb = consts.tile([F + 1, 4 * U], dt)
        for g in range(4):
            nc.sync.dma_start(out=wx_sb[:, g * U:(g + 1) * U], in_=wx_d[g])
        whe_sb = consts.tile([U, 4 * U], dt)
        nc.sync.dma_start(out=whe_sb, in_=whe_d)
        whd_sb = consts.tile([U, 4 * U], dt)
        nc.sync.dma_start(out=whd_sb, in_=whd_d)
        bdec_sb = consts.tile([4, U], dt)
        nc.sync.dma_start(out=bdec_sb, in_=bdec_d)
        maskc_sb = consts.tile([4, ZCH * 4 * BL], dt)
        nc.sync.dma_start(out=maskc_sb, in_=maskc_d)
        w1_sb = consts.tile([U, U], dt)
        nc.sync.dma_start(out=w1_sb, in_=w1_d)
        b1_sb = consts.tile([U, 1], dt)
        nc.sync.dma_start(out=b1_sb, in_=b1_d)
        w2_sb = consts.tile([U, F], dt)
        nc.sync.dma_start(out=w2_sb, in_=w2_d)
        b2t_sb = consts.tile([1, 8 * F], dt)
        nc.sync.dma_start(out=b2t_sb, in_=b2t_d)
        ones_sb = consts.tile([1, BL], dt)
        nc.sync.dma_start(out=ones_sb, in_=ones_d)
        zero_hc = consts.tile([U, BL], dt)
        nc.vector.memset(zero_hc, 0.0)

        # ---- x streamed in chunks ----
        xch = []
        for ci in range(NXC):
            xt = xpool.tile([F + 1, XCH, BL], dt, tag=f"x{ci}")
            nc.sync.dma_start(out=xt, in_=x_d[:, ci * XCH:(ci + 1) * XCH, :])
            xch.append(xt)

        seq_sb = seqp.tile([U, T_ * BL], dt)

        # ---- recurrence machinery ----
        z_tiles = {}

        def emit_xgemm(zc):
            """Encoder input projection (+bias via ones row) for z-chunk zc."""
            zt = zp.tile([U, 4, ZCH, BL], dt, tag="z")
            t0 = zc * ZCH
            xsl = xch[t0 // XCH][:, t0 % XCH:t0 % XCH + ZCH, :]
            xsl = xsl.rearrange("p a b -> p (a b)")
            for g in range(4):
                nc.tensor.matmul(zt[:, g, :, :].rearrange("p a b -> p (a b)"),
                                 lhsT=wx_sb[:, g * U:(g + 1) * U],
                                 rhs=xsl, start=(g == 0), stop=False,
                                 skip_group_check=True)
            z_tiles[zc] = zt

        def emit_bias_gemm(zc):
            """Decoder bias for z-chunk zc via mask matmul."""
            zt = zp.tile([U, 4, ZCH, BL], dt, tag="z")
            nc.tensor.matmul(
                zt[:, :, :, :].rearrange("p a b c -> p (a b c)"),
                lhsT=bdec_sb, rhs=maskc_sb, start=True, stop=False,
                skip_group_check=True)
            z_tiles[zc] = zt

        state = {"h": zero_hc, "c": zero_hc}

        def emit_step(t, wh_sb, dec):
            zt = z_tiles[t // ZCH]
            tl = t % ZCH
            h_prev, c_prev = state["h"], state["c"]
            for g in range(4):
                nc.tensor.matmul(zt[:, g, tl, :],
                                 lhsT=wh_sb[:, g * U:(g + 1) * U],
                                 rhs=h_prev, start=False,
                                 stop=(tl == ZCH - 1 and g == 3),
                                 skip_group_check=True)
            gsb = gp.tile([U, 4, BL], dt, tag="g")
            nc.scalar.activation(gsb, zt[:, :, tl, :], Sig)
            s_i, s_f, s_g, s_o = (gsb[:, k, :] for k in range(4))
            bt = tmp.tile([U, BL], dt, tag="bt")
            nc.vector.tensor_mul(bt, s_f, c_prev)
            ut = tmp.tile([U, BL], dt, tag="ut")
            nc.vector.scalar_tensor_tensor(ut, s_g, 0.5, s_i, sub, mul)
            c_new = cp.tile([U, BL], dt, tag="c")
            nc.vector.scalar_tensor_tensor(c_new, ut, 2.0, bt, mul, add)
            sc = scp.tile([U, BL], dt, tag="sc")
            nc.scalar.activation(sc, c_new, Sig, scale=2.0)
            if dec:
                h_new = seq_sb[:, t * BL:(t + 1) * BL]
            else:
                h_new = hp.tile([U, BL], dt, tag="h")
            nc.vector.scalar_tensor_tensor(h_new, sc, 0.5, s_o, sub, mul)
            state["h"], state["c"] = h_new, c_new

        # ---- encoder ----
        emit_xgemm(0)
        if NZ > 1:
            emit_xgemm(1)
        for zc in range(NZ):
            if zc + 2 < NZ:
                emit_xgemm(zc + 2)
            for tl in range(ZCH):
                emit_step(zc * ZCH + tl, whe_sb, dec=False)

        if dbg:
            nc.sync.dma_start(out=henc_d, in_=state["h"])
            nc.sync.dma_start(out=cenc_d, in_=state["c"])

        # ---- decoder (input == previous h, so only h-matmuls + bias) ----
        z_tiles.clear()
        emit_bias_gemm(0)
        if NZ > 1:
            emit_bias_gemm(1)
        for zc in range(NZ):
            if zc + 2 < NZ:
                emit_bias_gemm(zc + 2)
            for tl in range(ZCH):
                emit_step(zc * ZCH + tl, whd_sb, dec=True)

        if dbg:
            nc.sync.dma_start(out=seqdbg_d, in_=seq_sb)

        # ---- dense head: chunks of 8 timesteps ----
        for c8 in range(T_ // 8):
            hps = dps.tile([U, 8 * BL], dt, tag="hps")
            nc.tensor.matmul(hps, lhsT=w1_sb,
                             rhs=seq_sb[:, c8 * 8 * BL:(c8 + 1) * 8 * BL],
                             start=True, stop=True)
            hsb = dsb.tile([U, 8 * BL], dt, tag="hid")
            nc.scalar.activation(hsb, hps, Relu, bias=b1_sb)
            op = ops.tile([BL, 8 * F], dt, tag="op")
            for tl in range(8):
                nc.tensor.matmul(op[:, tl * F:(tl + 1) * F],
                                 lhsT=hsb[:, tl * BL:(tl + 1) * BL],
                                 rhs=w2_sb, start=(tl == 0), stop=False)
            nc.tensor.matmul(op, lhsT=ones_sb, rhs=b2t_sb,
                             start=False, stop=True)
            osb = dsb.tile([BL, 8 * F], dt, tag="osb")
            nc.scalar.copy(osb, op)
            nc.sync.dma_start(out=y_d[:, c8 * 8 * F:(c8 + 1) * 8 * F],
                              in_=osb)

    nc.compile()
    return nc


def _prepare_shared(enc_k, enc_rk, enc_b, dec_k, dec_rk, dec_b, w1, b1, w2,
                    b2):
    f32 = np.float32
    sg = np.array([1.0, 1.0, 2.0, 1.0], f32)   # Keras gate order i, f, g, o

    wx = np.empty((4, F + 1, U), f32)
    whe = np.empty((U, 4 * U), f32)
    whd = np.empty((U, 4 * U), f32)
    bdec = np.empty((4, U), f32)
    wdc = np.asarray(dec_k, f32) + np.asarray(dec_rk, f32)
    for g in range(4):
        sl = slice(g * U, (g + 1) * U)
        wx[g, :F, :] = np.asarray(enc_k, f32)[:, sl] * sg[g]
        wx[g, F, :] = np.asarray(enc_b, f32)[sl] * sg[g]
        whe[:, sl] = np.asarray(enc_rk, f32)[:, sl] * (2.0 * sg[g])
        whd[:, sl] = wdc[:, sl] * (2.0 * sg[g])
        bdec[g] = np.asarray(dec_b, f32)[sl] * sg[g]

    # z-chunk column order is (gate, tl, j) -> bias mask is block-diagonal
    maskc = np.kron(np.eye(4, dtype=f32), np.ones((1, ZCH * BL), f32))

    return {
        "wx": wx, "whe": whe, "whd": whd, "bdec": bdec, "maskc": maskc,
        "w1": (2.0 * np.asarray(w1, f32)),
        "b1": np.asarray(b1, f32).reshape(U, 1),
        "w2": np.asarray(w2, f32),
        "b2t": np.tile(np.asarray(b2, f32), 8).reshape(1, 8 * F),
        "ones": np.ones((1, BL), f32),
    }


def _prepare_host_inputs(input_tensor, **weights):
    shared = _prepare_shared(**weights)
    f32 = np.float32
    xt = np.ascontiguousarray(np.asarray(input_tensor, f32).transpose(2, 1, 0))
    t_len = xt.shape[1]
    in_maps = []
    for c in range(NCORES):
        xa = np.ones((F + 1, t_len, BL), f32)
        xa[:F] = xt[:, :, c * BL:(c + 1) * BL]
        in_maps.append({**shared, "x": xa})
    return in_maps


def _run(inputs, trace=False):
    from concourse import bass_utils
    if "nc" not in _CACHE:
        _CACHE["nc"] = _build_program()
    nc = _CACHE["nc"]
    in_maps = _prepare_host_inputs(**inputs)
    res = bass_utils.run_bass_kernel_spmd(nc, in_maps,
                                          core_ids=list(range(NCORES)),
                                          trace=trace)
    y = np.concatenate(
        [res.results[c]["y"].reshape(BL, T, F) for c in range(NCORES)], axis=0)
    return y.astype(np.float32), res


def kernel(**inputs):
    y, _ = _run(inputs)
    return y
